# revision 18
# baseline (speedup 1.0000x reference)
"""Distributed dot-product attention for TRN2, 8 NeuronCores.

Sharding: 8 cores = 4 batches x 2 head-groups (8 heads each).
Each core computes, for its (batch b, head-group g):
    Q = Xq[b] @ (Wq[g]/8).T ; K = Xk[b] @ Wk[g].T ; V = Xv[b] @ Wv[g].T
    per head h: A = exp(Q_h K_h^T); O_h = (A V_h) / rowsum(A)
    partial[b,g]^T = Wc[:, g] @ O^T                      (row-parallel)
Host: out[b] = (partial[b,0] + partial[b,1]).T + bc      (all-reduce + bias)

Device-side dataflow (all matmuls use the full 128-wide PE array):
  - S^T = K^T-stationary pairs on PE row-groups 0/64 (two heads concurrent).
  - exp(S^T) -> a_sb (bf16) on the scalar engine; this is the kernel's
    hard floor (~1.1us per [128,1024] tile) so everything else is
    scheduled into its slack.
  - AV uses a_sb chunks [128,128] as the *stationary* and V||ones [128,65]
    as the moving operand, accumulating O[q, d] + rowsum in PSUM across
    k-chunks.  The softmax denominator lands in column 64 of the
    accumulator, so normalization is a per-partition reciprocal +
    tensor_scalar multiply -- no partition broadcasts needed.
  - Normalized O pairs are transposed back to O^T pair tiles [128, seq]
    on the tensor engine (identity matmul), which makes composition a
    full-width contraction: partial^T[dout, q] += Wc-chunk.T @ O^T-pair.
  - The output is the *transposed* partial [D, seq]; the host transposes.

Scheduling: emission order = Tile scheduler priority.  The S^T/exp chain
is emitted first within each (pair, quarter) block; AV follows each exp;
projection and composition matmuls are queued as small filler units and
dripped one per lk-slot into the exp slack.
"""

import math
from collections import deque
from contextlib import ExitStack

import numpy as np
import ml_dtypes

import concourse.bass as bass
import concourse.bacc as bacc
import concourse.tile as tile
from concourse import mybir
from concourse import masks
from concourse.bass_utils import run_bass_kernel_spmd

B, L, D, H = 4, 2048, 1024, 16
DH = D // H          # 64 per-head dim
HPC = H // 2         # 8 heads per core
G = HPC * DH         # 512 head-group width
N_CORES = 8

f32 = mybir.dt.float32
bf16 = mybir.dt.bfloat16


def build_nc(seq=L, debug=False):
    """Build the per-core Bass program (SPMD, identical on all cores)."""
    KD = D // 128        # 8 contraction chunks over model dim
    LK = seq // 128      # Lk chunks (16)
    NPAIR = HPC // 2     # 4 head-pairs
    NQ = seq // 512      # Lq quarters (4)

    nc = bacc.Bacc(None, target_bir_lowering=False, debug=False)

    # X^T inputs arrive round-major: [n, 128, KD*512] -- each column
    # round is one [128, 4096] SBUF tile loaded by a single DMA whose
    # per-partition rows are contiguous 8KB segments (128 descriptors).
    xqT = nc.dram_tensor("xqT", [NQ, 128, KD * 512], bf16, kind="ExternalInput")
    xkT = nc.dram_tensor("xkT", [NQ, 128, KD * 512], bf16, kind="ExternalInput")
    xvT = nc.dram_tensor("xvT", [NQ, 128, KD * 512], bf16, kind="ExternalInput")
    wqT = nc.dram_tensor("wqT", [128, KD * G], bf16, kind="ExternalInput")
    wkT = nc.dram_tensor("wkT", [128, KD * G], bf16, kind="ExternalInput")
    wvT = nc.dram_tensor("wvT", [128, KD * G], bf16, kind="ExternalInput")
    wcT = nc.dram_tensor("wcT", [G, D], bf16, kind="ExternalInput")
    outp = nc.dram_tensor("outp", [2, D, seq], bf16, kind="ExternalOutput")
    dbg = {}
    if debug:
        for nm, shp, dt in [
                ("qt", [NPAIR, 128, seq], bf16), ("kt", [NPAIR, 128, seq], bf16),
                ("vt", [LK, 128, HPC, DH + 1], bf16),
                ("ot", [NPAIR, 128, seq], bf16),
                ("acc", [NQ * NPAIR, 128, 1024], f32)]:
            dbg[nm] = nc.dram_tensor(f"dbg_{nm}", shp, dt, kind="ExternalOutput")

    with tile.TileContext(nc) as tc, ExitStack() as ctx:
        Exp = mybir.ActivationFunctionType.Exp
        # Schraudolph exp on the DVE: bf16 bit pattern = round(x*SCH_A+SCH_B).
        # SCH_C calibrated so E[approx/exp] = 1 over the score distribution
        # (bias nulling -- the residual +-1.8% ripple averages out in softmax).
        SCH_A = 128.0 / math.log(2.0)
        SCH_B = 127.0 * 128.0 - 7.37
        DVE_LK = frozenset((3, 6, 9, 12, 15))
        DVE_LK_BY_P = (DVE_LK,) * 4

        # Persistent SBUF.
        const = ctx.enter_context(tc.tile_pool(name="const", bufs=1))
        # Q^T/K^T pair tiles rotate (pair-major: pair p's tiles are only
        # read during group p, and pair p+2's projection starts in group
        # p+1, after those reads are emitted).
        qkp = ctx.enter_context(tc.tile_pool(name="qkp", bufs=2))
        QT_d, KT_d = {}, {}

        def QT_t(p):
            if p not in QT_d:
                QT_d[p] = qkp.tile([128, seq], bf16, tag="qt", name=f"qt{p}")
            return QT_d[p]

        def KT_t(p):
            if p not in KT_d:
                KT_d[p] = qkp.tile([128, seq], bf16, tag="kt", name=f"kt{p}")
            return KT_d[p]
        V_t = [const.tile([128, HPC, DH + 1], bf16, tag=f"v{m}", name=f"v{m}")
               for m in range(LK)]
        # O^T pair tiles: head 2p on partitions 0:64, head 2p+1 on 64:128.
        OT_t = [const.tile([128, seq], bf16, tag=f"ot{p}", name=f"ot{p}")
                for p in range(NPAIR)]
        # Wc row chunks [g-chunk, D]: chunk gc rows are exactly pair gc's
        # (head, dh) rows, matching OT_t[gc]'s partition layout.
        wcB_t = [const.tile([128, D], bf16, tag=f"wc{gc}", name=f"wc{gc}")
                 for gc in range(NPAIR)]
        ident = const.tile([128, 128], bf16, tag="ident", name="ident")

        wpool = ctx.enter_context(tc.tile_pool(name="wpool", bufs=1))

        def load_w(src, pfx, pool=None):
            w_all = (pool or wpool).tile([128, KD * G], bf16, tag=pfx,
                                         name=pfx, bufs=1)
            nc.gpsimd.dma_start(out=w_all[:], in_=src[:])
            return [w_all[:, k * G:(k + 1) * G] for k in range(KD)]

        xcol = ctx.enter_context(tc.tile_pool(name="xcol", bufs=2))

        def load_xround(pfx, src, n, tag=None, bufs=None):
            t = xcol.tile([128, KD * 512], bf16, tag=tag or pfx,
                          name=f"{pfx}_{n}", bufs=bufs)
            nc.gpsimd.dma_start(out=t[:], in_=src[n])
            return [t[:, k * 512:(k + 1) * 512] for k in range(KD)]

        # PSUM: stp 2 tiles x 2 banks (4) + AV accumulator 2 banks
        # + gen (proj/comp/transpose) 2 x 1 bank = 8 banks exactly.
        stp_p = ctx.enter_context(
            tc.tile_pool(name="stp", bufs=2, space=bass.MemorySpace.PSUM))
        oap = ctx.enter_context(
            tc.tile_pool(name="oap", bufs=1, space=bass.MemorySpace.PSUM))
        gen_ps = ctx.enter_context(
            tc.tile_pool(name="gen_ps", bufs=2, space=bass.MemorySpace.PSUM))
        apool = ctx.enter_context(tc.tile_pool(name="apool", bufs=5))
        # Deferred a_sb tiles for the first block (its AV debt drains in
        # the second block, after V projection has streamed in).
        adef = ctx.enter_context(tc.tile_pool(name="adef", bufs=1))
        nrm = ctx.enter_context(tc.tile_pool(name="nrm", bufs=2))
        ost = ctx.enter_context(tc.tile_pool(name="ost", bufs=3))

        # ---- filler units: small closures emitting <=2 matmuls each,
        # dripped one per lk slot into the attention blocks' exp slack.
        # Each queue item is (key, emit_fn); drain_until(key) force-emits
        # everything up to `key` so producers always precede consumers.
        fillers = deque()

        def drain_until(key):
            while fillers and fillers[0][0] <= key:
                fillers.popleft()[1]()

        def pop_one(horizon=99):
            if fillers and fillers[0][0] <= horizon:
                fillers.popleft()[1]()

        def proj_round_units(w_t, xstate, dst, p, n):
            """One projection round (pair p, column round n) split into 4
            units of 2 chained matmuls; last unit evicts PSUM->dst.
            xstate is a dict whose "x" entry holds the 8 column tiles (may
            be filled later by a queued load unit)."""
            state = {}

            def unit(j):
                def emit():
                    if j == 0:
                        state["ps"] = gen_ps.tile([128, 512], f32, tag="pp",
                                                  name="pp")
                    ps = state["ps"]
                    for k in (2 * j, 2 * j + 1):
                        nc.tensor.matmul(
                            ps[:], lhsT=w_t[k][:, p * 128:(p + 1) * 128],
                            rhs=xstate["x"][k][:],
                            start=(k == 0), stop=(k == KD - 1))
                    if j == 3:
                        nc.vector.tensor_copy(
                            dst(p)[:, n * 512:(n + 1) * 512], ps[:])
                return emit
            return [unit(j) for j in range(4)]

        def comp_units(q, half, evict_scalar=False):
            """Half-composition for quarter q: pair-halves of the g
            contraction (gc in {2*half, 2*half+1}) into outp[half], so the
            first half streams out as soon as pair 1 is done."""
            qsl = slice(q * 512, (q + 1) * 512)
            units = []
            for dc in range(8):
                def unit(dc=dc):
                    def emit():
                        ps = gen_ps.tile([128, 512], f32, tag="pp", name="pp")
                        for j, gc in enumerate((2 * half, 2 * half + 1)):
                            nc.tensor.matmul(
                                ps[:], lhsT=wcB_t[gc][:, dc * 128:(dc + 1) * 128],
                                rhs=OT_t[gc][:, qsl],
                                start=(j == 0), stop=(j == 1))
                        o_sb = ost.tile([128, 512], bf16, tag="osb",
                                        name="osb")
                        # Alternate comp evictions scalar/vector so neither
                        # queue clogs ahead of the pipeline-critical exps.
                        if dc % 2 == 0:
                            nc.scalar.copy(o_sb[:], ps[:])
                        else:
                            nc.vector.tensor_copy(o_sb[:], ps[:])
                        nc.gpsimd.dma_start(
                            out=outp[half, dc * 128:(dc + 1) * 128, qsl],
                            in_=o_sb[:])
                    return emit
                units.append(unit())
            return units

        def proj_v_units(n, xstate, wv_t):
            """V projection round n as 16 filler units (2 matmuls each);
            every 4th unit evicts one V chunk."""
            units = []
            for mm in range(4):
                m = 4 * n + mm
                state = {}

                def unit(j, m=m, mm=mm, state=state):
                    def emit():
                        if j == 0:
                            state["ps"] = gen_ps.tile([128, G], f32,
                                                      tag="pp", name="pp")
                        ps = state["ps"]
                        for k in (2 * j, 2 * j + 1):
                            nc.tensor.matmul(
                                ps[:],
                                lhsT=xstate["x"][k][:, mm * 128:(mm + 1) * 128],
                                rhs=wv_t[k][:],
                                start=(k == 0), stop=(k == KD - 1))
                        if j == 3:
                            nc.vector.tensor_copy(
                                V_t[m][:, :, 0:DH],
                                ps[:].rearrange("p (h d) -> p h d", h=HPC))
                            nc.vector.memset(V_t[m][:, :, DH:DH + 1], 1.0)
                    return emit
                units += [unit(j) for j in range(4)]
            return units

        def attention_block(p, q, drain_key=0, pops=1, pending=None,
                            pending_slot=2, av_delay=3, defer_from=LK,
                            a_tiles=None, debt=None, horizon=99, pre=None,
                            adaptive=True):
            """S^T pair -> exp -> AV (a_sb stationary) for 16 lk chunks.

            Scheduling knobs (all in units of lk slots):
            - `pending` (previous block finalize: leftover AVs + normalize
              + transpose) fires at slot `pending_slot`, keeping this
              block's first S^Ts ahead of it in tensor priority.
            - own AV emission lags `av_delay` slots; the un-emitted tail
              is carried into the returned finalize closure.
            - `debt` (deferred AV units of an earlier block) drains 4 per
              slot over slots 0..3.
            - filler pops run from slot 3 on (after `pending`, so queued
              comp units never precede the OT tiles they read).
            With do_av=False the block emits no AV at all: a_tiles holds
            its a_sb tiles and the AV units are returned as `debt` for a
            later block.
            """
            drain_until(drain_key)
            qsl = slice(q * 512, (q + 1) * 512)
            # acc is allocated at its FIRST write (not block entry) so the
            # previous block's deferred writes/reads on the same single
            # PSUM buffer are all emitted before this generation begins.
            acc_h = {}

            def get_acc():
                if "t" not in acc_h:
                    acc_h["t"] = oap.tile([128, 1024], f32, tag="oacc",
                                          name="oacc")
                return acc_h["t"]

            def st_pair(lk):
                ksl = slice(lk * 128, (lk + 1) * 128)
                stp = stp_p.tile([128, 1024], f32, tag="stp", name="stp")
                nc.tensor.matmul(
                    stp[:, 0:512], lhsT=KT_t(p)[0:64, ksl],
                    rhs=QT_t(p)[0:64, qsl], start=True, stop=True,
                    tile_position=(0, 0))
                nc.tensor.matmul(
                    stp[:, 512:1024], lhsT=KT_t(p)[64:128, ksl],
                    rhs=QT_t(p)[64:128, qsl], start=True, stop=True,
                    tile_position=(64, 0))
                return stp

            def av_unit(lk, a_sb):
                def emit():
                    acc = get_acc()
                    # 8 AV matmuls: slot s = h*4 + qc at acc cols s*128
                    # (col s*128+64 accumulates the softmax denominator).
                    # start=True clears has_written for the WHOLE bank, so
                    # only the first group per bank (qc==0) starts; the
                    # other groups' first writes land on cleared bits and
                    # overwrite-then-set.
                    for h in range(2):
                        for qc in range(4):
                            s = h * 4 + qc
                            nc.tensor.matmul(
                                acc[:, s * 128:s * 128 + 65],
                                lhsT=a_sb[:, h * 512 + qc * 128:
                                          h * 512 + (qc + 1) * 128],
                                rhs=V_t[lk][:, 2 * p + h, :],
                                start=(lk == 0 and qc == 0),
                                stop=(lk == LK - 1))
                return emit

            debt_units = deque()
            avq = deque()
            stp = st_pair(0)
            for lk in range(LK):
                a_sb = (a_tiles[lk] if a_tiles is not None
                        and a_tiles[lk] is not None else
                        apool.tile([128, 1024], bf16, tag="a", name="a"))
                if lk in DVE_LK_BY_P[p]:
                    nc.vector.tensor_scalar(
                        out=a_sb[:].bitcast(mybir.dt.int16), in0=stp[:],
                        scalar1=SCH_A, scalar2=SCH_B,
                        op0=mybir.AluOpType.mult, op1=mybir.AluOpType.add)
                else:
                    nc.scalar.activation(a_sb[:], stp[:], Exp)
                # `pre` units (K rounds for THIS pair) must precede the
                # S^T emissions that read them; everything else fills in
                # after the S^T to keep the exp-feeding chain in front.
                if pre and lk in (0, 1, 4, 5, 8, 9):
                    for _ in range(2):
                        if pre:
                            pre.popleft()()
                if lk + 1 < LK:
                    stp = st_pair(lk + 1)
                if debt and lk < 3:
                    for _ in range(6):
                        if debt:
                            debt.popleft()()
                if lk == pending_slot and pending is not None:
                    pending()
                if lk >= 3:
                    n_pop = pops + (1 if adaptive and len(fillers) > 40
                                    else 0)
                    for _ in range(n_pop):
                        pop_one(horizon)
                u = av_unit(lk, a_sb)
                if lk < defer_from:
                    avq.append(u)
                    if lk >= av_delay:
                        avq.popleft()()
                else:
                    debt_units.append(u)

            def finalize():
                while avq:
                    avq.popleft()()
                acc = get_acc()
                if debug:
                    for sdbg in range(8):
                        atmp = nrm.tile([128, 128], f32, tag="dbga",
                                        name="dbga")
                        nc.vector.tensor_copy(
                            atmp[:], acc[:, sdbg * 128:(sdbg + 1) * 128])
                        nc.sync.dma_start(
                            out=dbg["acc"][q * NPAIR + p][:,
                                sdbg * 128:(sdbg + 1) * 128],
                            in_=atmp[:])
                # Fused normalize: ONE strided reciprocal (8 denominators at
                # acc col s*128+64) + ONE broadcast multiply -> onrm
                # [128, 8, 64] bf16.  Keeping this to 3 DVE ops (vs 16)
                # matters because the DVE also runs Schraudolph-exp tiles:
                # AV LDWEIGHTS stalls whenever finalize clogs the DVE queue.
                # onrm/tps are bf16 so the transpose matmul streams bf16.
                tps_f = gen_ps.tile([128, 512], f32, tag="pp", name="pp")
                tps = tps_f[:, 0:256].bitcast(bf16)
                rcp8 = nrm.tile([128, 8, 1], f32, tag="rcp8", name="rcp8")
                # onrm layout [q, qc, h, d] so each transpose qc reads a
                # contiguous [128, 128] stationary block.
                onrm = nrm.tile([128, 4, 2, 64], bf16, tag="on", name="on")
                acc3 = acc[:].rearrange("p (s c) -> p s c", s=8)
                nc.vector.reciprocal(out=rcp8[:], in_=acc3[:, :, 64:65])
                for h in range(2):
                    nc.vector.tensor_tensor(
                        out=onrm[:, :, h, :],
                        in0=acc3[:, h * 4:(h + 1) * 4, 0:64],
                        in1=rcp8[:, h * 4:(h + 1) * 4].broadcast_to(
                            (128, 4, 64)),
                        op=mybir.AluOpType.mult)
                # Transpose [q,dpair] -> [dpair,q] on the tensor engine.
                for qc in range(4):
                    nc.tensor.transpose(
                        tps[:, qc * 128:(qc + 1) * 128],
                        onrm[:, qc, :, :], ident[:])
                nc.vector.tensor_copy(OT_t[p][:, qsl], tps[:])
            return finalize, debt_units

        # ---------------- emission (order = priority) ----------------
        # Pair-major block order: group p = blocks (p, q=0..3).  K pair
        # p+1 and Q rounds for pair p drip into group p's slack, so the
        # projection load spreads over the whole run instead of piling
        # into quarter 0.  DMA issues are need-ordered (FIFO transfers).
        wk_t = load_w(wkT, "wk")
        xk_r = {0: {"x": load_xround("xk", xkT, 0, tag="xk0", bufs=1)}}
        wq_t = load_w(wqT, "wq")
        xq_r = {0: {"x": load_xround("xq", xqT, 0, tag="xq0", bufs=1)},
                1: {}, 2: {}, 3: {}}
        xk_r[1] = {"x": load_xround("xk", xkT, 1, tag="xk1", bufs=1)}
        wv_t = load_w(wvT, "wv")
        xv_r = {0: {}, 1: {}, 2: {}, 3: {}}
        xv_r[0]["x"] = load_xround("xv", xvT, 0)
        xk_r[2] = {"x": load_xround("xk", xkT, 2, tag="xk2", bufs=1)}
        xk_r[3] = {"x": load_xround("xk", xkT, 3, tag="xk3", bufs=1)}
        xv_r[1]["x"] = load_xround("xv", xvT, 1)
        masks.make_identity(nc, ident[:])
        adef_t = [adef.tile([128, 1024], bf16, tag=f"ad{i}", name=f"ad{i}")
                  for i in range(LK)]

        for u in proj_round_units(wk_t, xk_r[0], KT_t, 0, 0):
            u()
        for u in proj_round_units(wq_t, xq_r[0], QT_t, 0, 0):
            u()

        # Filler queue (keys must be queued in sorted order).  Key map:
        # K pair p + Q pair p round 0 -> 4p; Q pair p round q -> 4p+q;
        # V and K0 r1-3 -> 0; wc -> 16; comp(q) -> 17+q.
        pre_k0 = deque()
        for n in range(1, NQ):
            for u in proj_round_units(wk_t, xk_r[n], KT_t, 0, n):
                pre_k0.append(u)
        def loadq(r):
            def emit():
                xq_r[r]["x"] = load_xround("xq", xqT, r, tag=f"xq{r}",
                                           bufs=1)
            return emit
        fillers.append((0, loadq(1)))
        for n in range(NQ):
            if n == 1:
                def loadv2():
                    xv_r[2]["x"] = load_xround("xv", xvT, 2)
                fillers.append((0, loadv2))
            if n == 2:
                def loadv3():
                    xv_r[3]["x"] = load_xround("xv", xvT, 3)
                fillers.append((0, loadv3))
            for u in proj_v_units(n, xv_r[n], wv_t):
                fillers.append((0, u))
        for q in range(1, NQ):
            if q + 1 < NQ:
                fillers.append((q - 1 if q > 1 else 1, loadq(q + 1)))
            for u in proj_round_units(wq_t, xq_r[q], QT_t, 0, q):
                fillers.append((q, u))
        for p in range(1, NPAIR):
            for n in range(NQ):
                for u in proj_round_units(wk_t, xk_r[n], KT_t, p, n):
                    fillers.append((4 * p, u))
            for u in proj_round_units(wq_t, xq_r[0], QT_t, p, 0):
                fillers.append((4 * p, u))
            for q in range(1, NQ):
                for u in proj_round_units(wq_t, xq_r[q], QT_t, p, q):
                    fillers.append((4 * p + q, u))

        def load_wc():
            for gc in range(NPAIR):
                nc.gpsimd.dma_start(out=wcB_t[gc][:],
                                    in_=wcT[gc * 128:(gc + 1) * 128, :])
        fillers.append((12, load_wc))


        fin = None
        debt = None
        for p in range(NPAIR):
            for q in range(NQ):
                first = (p == 0 and q == 0)
                second = (p == 0 and q == 1)
                last = (p == NPAIR - 1 and q == NQ - 1)
                fin, du = attention_block(
                    p, q, drain_key=4 * p + q,
                    pops=(4 if p == 0 else (2 if p >= 2 else 1)),
                    pending=fin,
                    pending_slot=(3 if second else 2),
                    av_delay=(4 if second else (1 if last else 3)),
                    defer_from=(0 if first else LK),
                    a_tiles=(adef_t if first else None),
                    debt=(debt if second else None),
                    horizon=4 * p + 7,
                    pre=(pre_k0 if first else None))
                if first:
                    debt = du
                if p == 1:
                    for u in comp_units(q, 0):
                        fillers.append((15, u))
                if p == NPAIR - 1 and q + 1 < NQ:
                    for u in comp_units(q, 1):
                        fillers.append((19, u))
        drain_until(99)
        fin()
        for u in comp_units(NQ - 1, 1, evict_scalar=True):
            u()

        if debug:
            for p in range(NPAIR):
                nc.sync.dma_start(out=dbg["ot"][p], in_=OT_t[p][:])
            for m in range(LK):
                nc.sync.dma_start(out=dbg["vt"][m], in_=V_t[m][:])

    nc.compile()
    return nc


def shard_inputs(keys, queries, values, Wk, Wq, Wv, Wc, seq=L):
    """Host-side shard prep: per-core transposed bf16 operands."""

    def bf(a):
        return np.ascontiguousarray(a).astype(ml_dtypes.bfloat16)

    def bft(x):
        # [seq, D] -> X^T round-major [n, 128, KD*512]: partition p of
        # round n holds the 8 k-chunk rows contiguously (8KB DMA rows).
        xt = np.ascontiguousarray(x.T).astype(ml_dtypes.bfloat16)
        kd, nq = xt.shape[0] // 128, xt.shape[1] // 512
        return np.ascontiguousarray(
            xt.reshape(kd, 128, nq, 512).transpose(2, 1, 0, 3)
        ).reshape(nq, 128, kd * 512)

    def wblk(w):
        # [D, G] -> [128, (D//128)*G] with k-block at cols k*G:(k+1)*G
        return bf(np.ascontiguousarray(
            np.asarray(w).reshape(D // 128, 128, G).transpose(1, 0, 2)
        ).reshape(128, (D // 128) * G))

    scale = 1.0 / math.sqrt(DH)
    in_maps = []
    for c in range(N_CORES):
        b, g = c // 2, c % 2
        gs = slice(g * G, (g + 1) * G)
        in_maps.append({
            "xqT": bft(queries[b, :seq]),
            "xkT": bft(keys[b, :seq]),
            "xvT": bft(values[b, :seq]),
            "wqT": wblk(Wq[gs, :].T * scale),
            "wkT": wblk(Wk[gs, :].T),
            "wvT": wblk(Wv[gs, :].T),
            "wcT": bf(Wc[:, gs].T),
        })
    return in_maps


_NC_CACHE = {}


def run_cores(inputs, seq=L, trace=False, debug=False):
    key = (seq, debug)
    if key not in _NC_CACHE:
        _NC_CACHE[key] = build_nc(seq, debug=debug)
    nc = _NC_CACHE[key]
    in_maps = shard_inputs(
        inputs["keys"], inputs["queries"], inputs["values"],
        inputs["Wk"], inputs["Wq"], inputs["Wv"], inputs["Wc"], seq=seq)
    res = run_bass_kernel_spmd(nc, in_maps, core_ids=list(range(N_CORES)),
                               trace=trace)
    return res


def kernel(keys, queries, values, Wk, Wq, Wv, Wc, bc, attn_mask):
    res = run_cores(dict(keys=np.asarray(keys), queries=np.asarray(queries),
                         values=np.asarray(values), Wk=np.asarray(Wk),
                         Wq=np.asarray(Wq), Wv=np.asarray(Wv),
                         Wc=np.asarray(Wc)))
    bc = np.asarray(bc, np.float32)
    out = np.empty((B, L, D), np.float32)
    for b in range(B):
        out[b] = (res.results[2 * b]["outp"].astype(np.float32).sum(0)
                  + res.results[2 * b + 1]["outp"].astype(np.float32).sum(0)
                  ).T + bc
    return out



# revision 19
# speedup vs baseline: 1.2025x; 1.2025x over previous
"""Distributed dot-product attention for TRN2, 8 NeuronCores.

Sharding: 8 cores = 4 batches x 2 head-groups (8 heads each).
Each core computes, for its (batch b, head-group g):
    Q = Xq[b] @ (Wq[g]/8).T ; K = Xk[b] @ Wk[g].T ; V = Xv[b] @ Wv[g].T
    per head h: A = exp(Q_h K_h^T); O_h = (A V_h) / rowsum(A)
    partial[b,g]^T = Wc[:, g] @ O^T                      (row-parallel)
Host: out[b] = (partial[b,0] + partial[b,1]).T + bc      (all-reduce + bias)

Device-side dataflow (all matmuls use the full 128-wide PE array):
  - S^T = K^T-stationary pairs on PE row-groups 0/64 (two heads concurrent).
  - exp(S^T) -> a_sb (bf16) on the scalar engine; this is the kernel's
    hard floor (~1.1us per [128,1024] tile) so everything else is
    scheduled into its slack.
  - AV uses a_sb chunks [128,128] as the *stationary* and V||ones [128,65]
    as the moving operand, accumulating O[q, d] + rowsum in PSUM across
    k-chunks.  The softmax denominator lands in column 64 of the
    accumulator, so normalization is a per-partition reciprocal +
    tensor_scalar multiply -- no partition broadcasts needed.
  - Normalized O pairs are transposed back to O^T pair tiles [128, seq]
    on the tensor engine (identity matmul), which makes composition a
    full-width contraction: partial^T[dout, q] += Wc-chunk.T @ O^T-pair.
  - The output is the *transposed* partial [D, seq]; the host transposes.

Scheduling: emission order = Tile scheduler priority.  The S^T/exp chain
is emitted first within each (pair, quarter) block; AV follows each exp;
projection and composition matmuls are queued as small filler units and
dripped one per lk-slot into the exp slack.
"""

import math
from collections import deque
from contextlib import ExitStack

import numpy as np
import ml_dtypes

import concourse.bass as bass
import concourse.bacc as bacc
import concourse.tile as tile
from concourse import mybir
from concourse import masks
from concourse.bass_utils import run_bass_kernel_spmd

B, L, D, H = 4, 2048, 1024, 16
DH = D // H          # 64 per-head dim
HPC = H // 2         # 8 heads per core
G = HPC * DH         # 512 head-group width
N_CORES = 8

f32 = mybir.dt.float32
bf16 = mybir.dt.bfloat16


def build_nc(seq=L, debug=False):
    """Build the per-core Bass program (SPMD, identical on all cores)."""
    KD = D // 128        # 8 contraction chunks over model dim
    LK = seq // 128      # Lk chunks (16)
    NPAIR = HPC // 2     # 4 head-pairs
    NQ = seq // 512      # Lq quarters (4)

    nc = bacc.Bacc(None, target_bir_lowering=False, debug=False)

    # X^T inputs arrive round-major: [n, 128, KD*512] -- each column
    # round is one [128, 4096] SBUF tile loaded by a single DMA whose
    # per-partition rows are contiguous 8KB segments (128 descriptors).
    xqT = nc.dram_tensor("xqT", [NQ, 128, KD * 512], bf16, kind="ExternalInput")
    xkT = nc.dram_tensor("xkT", [NQ, 128, KD * 512], bf16, kind="ExternalInput")
    xvT = nc.dram_tensor("xvT", [NQ, 128, KD * 512], bf16, kind="ExternalInput")
    wqT = nc.dram_tensor("wqT", [128, KD * G], bf16, kind="ExternalInput")
    wkT = nc.dram_tensor("wkT", [128, KD * G], bf16, kind="ExternalInput")
    wvT = nc.dram_tensor("wvT", [128, KD * G], bf16, kind="ExternalInput")
    wcT = nc.dram_tensor("wcT", [G, D], bf16, kind="ExternalInput")
    outp = nc.dram_tensor("outp", [2, D, seq], bf16, kind="ExternalOutput")
    dbg = {}
    if debug:
        for nm, shp, dt in [
                ("qt", [NPAIR, 128, seq], bf16), ("kt", [NPAIR, 128, seq], bf16),
                ("vt", [LK, 128, HPC, DH + 1], bf16),
                ("ot", [NPAIR, 128, seq], bf16),
                ("acc", [NQ * NPAIR, 128, 1024], f32)]:
            dbg[nm] = nc.dram_tensor(f"dbg_{nm}", shp, dt, kind="ExternalOutput")

    with tile.TileContext(nc) as tc, ExitStack() as ctx:
        Exp = mybir.ActivationFunctionType.Exp
        # Schraudolph exp on the DVE: bf16 bit pattern = round(x*SCH_A+SCH_B).
        # SCH_C calibrated so E[approx/exp] = 1 over the score distribution
        # (bias nulling -- the residual +-1.8% ripple averages out in softmax).
        SCH_A = 128.0 / math.log(2.0)
        SCH_B = 127.0 * 128.0 - 7.37
        DVE_LK = frozenset((3, 6, 9, 12, 15))
        DVE_LK_BY_P = (DVE_LK,) * 4

        # Persistent SBUF.
        const = ctx.enter_context(tc.tile_pool(name="const", bufs=1))
        # Q^T/K^T pair tiles rotate (pair-major: pair p's tiles are only
        # read during group p, and pair p+2's projection starts in group
        # p+1, after those reads are emitted).
        qkp = ctx.enter_context(tc.tile_pool(name="qkp", bufs=2))
        QT_d, KT_d = {}, {}

        def QT_t(p):
            if p not in QT_d:
                QT_d[p] = qkp.tile([128, seq], bf16, tag="qt", name=f"qt{p}")
            return QT_d[p]

        def KT_t(p):
            if p not in KT_d:
                KT_d[p] = qkp.tile([128, seq], bf16, tag="kt", name=f"kt{p}")
            return KT_d[p]
        V_t = [const.tile([128, HPC, DH + 1], bf16, tag=f"v{m}", name=f"v{m}")
               for m in range(LK)]
        # O^T pair tiles: head 2p on partitions 0:64, head 2p+1 on 64:128.
        OT_t = [const.tile([128, seq], bf16, tag=f"ot{p}", name=f"ot{p}")
                for p in range(NPAIR)]
        # Wc row chunks [g-chunk, D]: chunk gc rows are exactly pair gc's
        # (head, dh) rows, matching OT_t[gc]'s partition layout.
        wcB_t = [const.tile([128, D], bf16, tag=f"wc{gc}", name=f"wc{gc}")
                 for gc in range(NPAIR)]
        ident = const.tile([128, 128], bf16, tag="ident", name="ident")

        wpool = ctx.enter_context(tc.tile_pool(name="wpool", bufs=1))

        def load_w(src, pfx, pool=None):
            w_all = (pool or wpool).tile([128, KD * G], bf16, tag=pfx,
                                         name=pfx, bufs=1)
            nc.gpsimd.dma_start(out=w_all[:], in_=src[:])
            return [w_all[:, k * G:(k + 1) * G] for k in range(KD)]

        xcol = ctx.enter_context(tc.tile_pool(name="xcol", bufs=2))

        def load_xround(pfx, src, n, tag=None, bufs=None):
            t = xcol.tile([128, KD * 512], bf16, tag=tag or pfx,
                          name=f"{pfx}_{n}", bufs=bufs)
            nc.gpsimd.dma_start(out=t[:], in_=src[n])
            return [t[:, k * 512:(k + 1) * 512] for k in range(KD)]

        # PSUM: stp 2 tiles x 2 banks (4) + AV accumulator 2 banks
        # + gen (proj/comp/transpose) 2 x 1 bank = 8 banks exactly.
        stp_p = ctx.enter_context(
            tc.tile_pool(name="stp", bufs=2, space=bass.MemorySpace.PSUM))
        oap = ctx.enter_context(
            tc.tile_pool(name="oap", bufs=1, space=bass.MemorySpace.PSUM))
        gen_ps = ctx.enter_context(
            tc.tile_pool(name="gen_ps", bufs=2, space=bass.MemorySpace.PSUM))
        apool = ctx.enter_context(tc.tile_pool(name="apool", bufs=5))
        # Deferred a_sb tiles for the first block (its AV debt drains in
        # the second block, after V projection has streamed in).
        adef = ctx.enter_context(tc.tile_pool(name="adef", bufs=1))
        nrm = ctx.enter_context(tc.tile_pool(name="nrm", bufs=2))
        ost = ctx.enter_context(tc.tile_pool(name="ost", bufs=3))

        # ---- filler units: small closures emitting <=2 matmuls each,
        # dripped one per lk slot into the attention blocks' exp slack.
        # Each queue item is (key, emit_fn); drain_until(key) force-emits
        # everything up to `key` so producers always precede consumers.
        fillers = deque()

        def drain_until(key):
            while fillers and fillers[0][0] <= key:
                fillers.popleft()[1]()

        def pop_one(horizon=99):
            if fillers and fillers[0][0] <= horizon:
                fillers.popleft()[1]()

        def proj_round_units(w_t, xstate, dst, p, n):
            """One projection round (pair p, column round n) split into 4
            units of 2 chained matmuls; last unit evicts PSUM->dst.
            xstate is a dict whose "x" entry holds the 8 column tiles (may
            be filled later by a queued load unit)."""
            state = {}

            def unit(j):
                def emit():
                    if j == 0:
                        state["ps"] = gen_ps.tile([128, 512], f32, tag="pp",
                                                  name="pp")
                    ps = state["ps"]
                    for k in (2 * j, 2 * j + 1):
                        nc.tensor.matmul(
                            ps[:], lhsT=w_t[k][:, p * 128:(p + 1) * 128],
                            rhs=xstate["x"][k][:],
                            start=(k == 0), stop=(k == KD - 1))
                    if j == 3:
                        nc.vector.tensor_copy(
                            dst(p)[:, n * 512:(n + 1) * 512], ps[:])
                return emit
            return [unit(j) for j in range(4)]

        def comp_units(q, half, evict_scalar=False):
            """Half-composition for quarter q: pair-halves of the g
            contraction (gc in {2*half, 2*half+1}) into outp[half], so the
            first half streams out as soon as pair 1 is done."""
            qsl = slice(q * 512, (q + 1) * 512)
            units = []
            for dc in range(8):
                def unit(dc=dc):
                    def emit():
                        ps = gen_ps.tile([128, 512], f32, tag="pp", name="pp")
                        for j, gc in enumerate((2 * half, 2 * half + 1)):
                            nc.tensor.matmul(
                                ps[:], lhsT=wcB_t[gc][:, dc * 128:(dc + 1) * 128],
                                rhs=OT_t[gc][:, qsl],
                                start=(j == 0), stop=(j == 1))
                        o_sb = ost.tile([128, 512], bf16, tag="osb",
                                        name="osb")
                        # Alternate comp evictions scalar/vector so neither
                        # queue clogs ahead of the pipeline-critical exps.
                        if dc % 2 == 0:
                            nc.scalar.copy(o_sb[:], ps[:])
                        else:
                            nc.vector.tensor_copy(o_sb[:], ps[:])
                        nc.gpsimd.dma_start(
                            out=outp[half, dc * 128:(dc + 1) * 128, qsl],
                            in_=o_sb[:])
                    return emit
                units.append(unit())
            return units

        def proj_v_units(n, xstate, wv_t):
            """V projection round n as 16 filler units (2 matmuls each);
            every 4th unit evicts one V chunk."""
            units = []
            for mm in range(4):
                m = 4 * n + mm
                state = {}

                def unit(j, m=m, mm=mm, state=state):
                    def emit():
                        if j == 0:
                            state["ps"] = gen_ps.tile([128, G], f32,
                                                      tag="pp", name="pp")
                        ps = state["ps"]
                        for k in (2 * j, 2 * j + 1):
                            nc.tensor.matmul(
                                ps[:],
                                lhsT=xstate["x"][k][:, mm * 128:(mm + 1) * 128],
                                rhs=wv_t[k][:],
                                start=(k == 0), stop=(k == KD - 1))
                        if j == 3:
                            nc.vector.tensor_copy(
                                V_t[m][:, :, 0:DH],
                                ps[:].rearrange("p (h d) -> p h d", h=HPC))
                            nc.vector.memset(V_t[m][:, :, DH:DH + 1], 1.0)
                    return emit
                units += [unit(j) for j in range(4)]
            return units

        def attention_block(p, q, drain_key=0, pops=1, pending=None,
                            pending_slot=2, av_delay=3, defer_from=LK,
                            a_tiles=None, debt=None, horizon=99, pre=None,
                            adaptive=True):
            """S^T pair -> exp -> AV (a_sb stationary) for 16 lk chunks.

            Scheduling knobs (all in units of lk slots):
            - `pending` (previous block finalize: leftover AVs + normalize
              + transpose) fires at slot `pending_slot`, keeping this
              block's first S^Ts ahead of it in tensor priority.
            - own AV emission lags `av_delay` slots; the un-emitted tail
              is carried into the returned finalize closure.
            - `debt` (deferred AV units of an earlier block) drains 4 per
              slot over slots 0..3.
            - filler pops run from slot 3 on (after `pending`, so queued
              comp units never precede the OT tiles they read).
            With do_av=False the block emits no AV at all: a_tiles holds
            its a_sb tiles and the AV units are returned as `debt` for a
            later block.
            """
            drain_until(drain_key)
            qsl = slice(q * 512, (q + 1) * 512)
            # acc is allocated at its FIRST write (not block entry) so the
            # previous block's deferred writes/reads on the same single
            # PSUM buffer are all emitted before this generation begins.
            acc_h = {}

            def get_acc():
                if "t" not in acc_h:
                    acc_h["t"] = oap.tile([128, 1024], f32, tag="oacc",
                                          name="oacc")
                return acc_h["t"]

            def st_pair(lk):
                ksl = slice(lk * 128, (lk + 1) * 128)
                stp = stp_p.tile([128, 1024], f32, tag="stp", name="stp")
                nc.tensor.matmul(
                    stp[:, 0:512], lhsT=KT_t(p)[0:64, ksl],
                    rhs=QT_t(p)[0:64, qsl], start=True, stop=True,
                    tile_position=(0, 0))
                nc.tensor.matmul(
                    stp[:, 512:1024], lhsT=KT_t(p)[64:128, ksl],
                    rhs=QT_t(p)[64:128, qsl], start=True, stop=True,
                    tile_position=(64, 0))
                return stp

            def av_unit(lk, a_sb):
                def emit():
                    acc = get_acc()
                    # 8 AV matmuls: slot s = h*4 + qc at acc cols s*128
                    # (col s*128+64 accumulates the softmax denominator).
                    # start=True clears has_written for the WHOLE bank, so
                    # only the first group per bank (qc==0) starts; the
                    # other groups' first writes land on cleared bits and
                    # overwrite-then-set.
                    for h in range(2):
                        for qc in range(4):
                            s = h * 4 + qc
                            nc.tensor.matmul(
                                acc[:, s * 128:s * 128 + 65],
                                lhsT=a_sb[:, h * 512 + qc * 128:
                                          h * 512 + (qc + 1) * 128],
                                rhs=V_t[lk][:, 2 * p + h, :],
                                start=(lk == 0 and qc == 0),
                                stop=(lk == LK - 1))
                return emit

            debt_units = deque()
            avq = deque()
            stp = st_pair(0)
            for lk in range(LK):
                a_sb = (a_tiles[lk] if a_tiles is not None
                        and a_tiles[lk] is not None else
                        apool.tile([128, 1024], bf16, tag="a", name="a"))
                if lk in DVE_LK_BY_P[p]:
                    nc.vector.tensor_scalar(
                        out=a_sb[:].bitcast(mybir.dt.int16), in0=stp[:],
                        scalar1=SCH_A, scalar2=SCH_B,
                        op0=mybir.AluOpType.mult, op1=mybir.AluOpType.add)
                else:
                    nc.scalar.activation(a_sb[:], stp[:], Exp)
                # `pre` units (K rounds for THIS pair) must precede the
                # S^T emissions that read them; everything else fills in
                # after the S^T to keep the exp-feeding chain in front.
                if pre and lk in (0, 1, 4, 5, 8, 9):
                    for _ in range(2):
                        if pre:
                            pre.popleft()()
                if lk + 1 < LK:
                    stp = st_pair(lk + 1)
                if debt and lk < 3:
                    for _ in range(6):
                        if debt:
                            debt.popleft()()
                if lk == pending_slot and pending is not None:
                    pending()
                if lk >= 3:
                    n_pop = pops + (1 if adaptive and len(fillers) > 40
                                    else 0)
                    for _ in range(n_pop):
                        pop_one(horizon)
                u = av_unit(lk, a_sb)
                if lk < defer_from:
                    avq.append(u)
                    if lk >= av_delay:
                        avq.popleft()()
                else:
                    debt_units.append(u)

            def finalize():
                while avq:
                    avq.popleft()()
                acc = get_acc()
                if debug:
                    for sdbg in range(8):
                        atmp = nrm.tile([128, 128], f32, tag="dbga",
                                        name="dbga")
                        nc.vector.tensor_copy(
                            atmp[:], acc[:, sdbg * 128:(sdbg + 1) * 128])
                        nc.sync.dma_start(
                            out=dbg["acc"][q * NPAIR + p][:,
                                sdbg * 128:(sdbg + 1) * 128],
                            in_=atmp[:])
                # Fused normalize: ONE strided reciprocal (8 denominators at
                # acc col s*128+64) + ONE broadcast multiply -> onrm
                # [128, 8, 64] bf16.  Keeping this to 3 DVE ops (vs 16)
                # matters because the DVE also runs Schraudolph-exp tiles:
                # AV LDWEIGHTS stalls whenever finalize clogs the DVE queue.
                # onrm/tps are bf16 so the transpose matmul streams bf16.
                tps_f = gen_ps.tile([128, 512], f32, tag="pp", name="pp")
                tps = tps_f[:, 0:256].bitcast(bf16)
                rcp8 = nrm.tile([128, 8, 1], f32, tag="rcp8", name="rcp8")
                # onrm layout [q, qc, h, d] so each transpose qc reads a
                # contiguous [128, 128] stationary block.
                onrm = nrm.tile([128, 4, 2, 64], bf16, tag="on", name="on")
                acc3 = acc[:].rearrange("p (s c) -> p s c", s=8)
                nc.vector.reciprocal(out=rcp8[:], in_=acc3[:, :, 64:65])
                for h in range(2):
                    nc.vector.tensor_tensor(
                        out=onrm[:, :, h, :],
                        in0=acc3[:, h * 4:(h + 1) * 4, 0:64],
                        in1=rcp8[:, h * 4:(h + 1) * 4].broadcast_to(
                            (128, 4, 64)),
                        op=mybir.AluOpType.mult)
                # Transpose [q,dpair] -> [dpair,q] on the tensor engine.
                for qc in range(4):
                    nc.tensor.transpose(
                        tps[:, qc * 128:(qc + 1) * 128],
                        onrm[:, qc, :, :], ident[:])
                nc.vector.tensor_copy(OT_t[p][:, qsl], tps[:])
            return finalize, debt_units

        # ---------------- emission (order = priority) ----------------
        # Pair-major block order: group p = blocks (p, q=0..3).  K pair
        # p+1 and Q rounds for pair p drip into group p's slack, so the
        # projection load spreads over the whole run instead of piling
        # into quarter 0.  DMA issues are need-ordered (FIFO transfers).
        wk_t = load_w(wkT, "wk")
        xk_r = {0: {"x": load_xround("xk", xkT, 0, tag="xk0", bufs=1)}}
        wq_t = load_w(wqT, "wq")
        xq_r = {0: {"x": load_xround("xq", xqT, 0, tag="xq0", bufs=1)},
                1: {}, 2: {}, 3: {}}
        xk_r[1] = {"x": load_xround("xk", xkT, 1, tag="xk1", bufs=1)}
        wv_t = load_w(wvT, "wv")
        xv_r = {0: {}, 1: {}, 2: {}, 3: {}}
        xv_r[0]["x"] = load_xround("xv", xvT, 0)
        xk_r[2] = {"x": load_xround("xk", xkT, 2, tag="xk2", bufs=1)}
        xk_r[3] = {"x": load_xround("xk", xkT, 3, tag="xk3", bufs=1)}
        xv_r[1]["x"] = load_xround("xv", xvT, 1)
        masks.make_identity(nc, ident[:])
        adef_t = [adef.tile([128, 1024], bf16, tag=f"ad{i}", name=f"ad{i}")
                  for i in range(LK)]

        for u in proj_round_units(wk_t, xk_r[0], KT_t, 0, 0):
            u()
        for u in proj_round_units(wq_t, xq_r[0], QT_t, 0, 0):
            u()

        # Filler queue (keys must be queued in sorted order).  Key map:
        # K pair p + Q pair p round 0 -> 4p; Q pair p round q -> 4p+q;
        # V and K0 r1-3 -> 0; wc -> 16; comp(q) -> 17+q.
        pre_k0 = deque()
        for n in range(1, NQ):
            for u in proj_round_units(wk_t, xk_r[n], KT_t, 0, n):
                pre_k0.append(u)
        def loadq(r):
            def emit():
                xq_r[r]["x"] = load_xround("xq", xqT, r, tag=f"xq{r}",
                                           bufs=1)
            return emit
        fillers.append((0, loadq(1)))
        for n in range(NQ):
            if n == 1:
                def loadv2():
                    xv_r[2]["x"] = load_xround("xv", xvT, 2)
                fillers.append((0, loadv2))
            if n == 2:
                def loadv3():
                    xv_r[3]["x"] = load_xround("xv", xvT, 3)
                fillers.append((0, loadv3))
            for u in proj_v_units(n, xv_r[n], wv_t):
                fillers.append((0, u))
        for q in range(1, NQ):
            if q + 1 < NQ:
                fillers.append((q - 1 if q > 1 else 1, loadq(q + 1)))
            for u in proj_round_units(wq_t, xq_r[q], QT_t, 0, q):
                fillers.append((q, u))
        for p in range(1, NPAIR):
            for n in range(NQ):
                for u in proj_round_units(wk_t, xk_r[n], KT_t, p, n):
                    fillers.append((4 * p, u))
            for u in proj_round_units(wq_t, xq_r[0], QT_t, p, 0):
                fillers.append((4 * p, u))
            for q in range(1, NQ):
                for u in proj_round_units(wq_t, xq_r[q], QT_t, p, q):
                    fillers.append((4 * p + q, u))

        def load_wc():
            for gc in range(NPAIR):
                nc.gpsimd.dma_start(out=wcB_t[gc][:],
                                    in_=wcT[gc * 128:(gc + 1) * 128, :])
        fillers.append((12, load_wc))


        fin = None
        debt = None
        for p in range(NPAIR):
            for q in range(NQ):
                first = (p == 0 and q == 0)
                second = (p == 0 and q == 1)
                last = (p == NPAIR - 1 and q == NQ - 1)
                fin, du = attention_block(
                    p, q, drain_key=4 * p + q,
                    pops=(4 if p == 0 else (2 if last else 1)),
                    pending=fin,
                    pending_slot=(3 if second else 2),
                    av_delay=(4 if second else (1 if last else 3)),
                    defer_from=(0 if first else LK),
                    a_tiles=(adef_t if first else None),
                    debt=(debt if second else None),
                    horizon=4 * p + 7,
                    pre=(pre_k0 if first else None))
                if first:
                    debt = du
                if p == 1:
                    for u in comp_units(q, 0):
                        fillers.append((15, u))
                if p == NPAIR - 1 and q + 1 < NQ:
                    for u in comp_units(q, 1):
                        fillers.append((19, u))
        drain_until(99)
        fin()
        for u in comp_units(NQ - 1, 1, evict_scalar=True):
            u()

        if debug:
            for p in range(NPAIR):
                nc.sync.dma_start(out=dbg["ot"][p], in_=OT_t[p][:])
            for m in range(LK):
                nc.sync.dma_start(out=dbg["vt"][m], in_=V_t[m][:])

    nc.compile()
    return nc


def shard_inputs(keys, queries, values, Wk, Wq, Wv, Wc, seq=L):
    """Host-side shard prep: per-core transposed bf16 operands."""

    def bf(a):
        return np.ascontiguousarray(a).astype(ml_dtypes.bfloat16)

    def bft(x):
        # [seq, D] -> X^T round-major [n, 128, KD*512]: partition p of
        # round n holds the 8 k-chunk rows contiguously (8KB DMA rows).
        xt = np.ascontiguousarray(x.T).astype(ml_dtypes.bfloat16)
        kd, nq = xt.shape[0] // 128, xt.shape[1] // 512
        return np.ascontiguousarray(
            xt.reshape(kd, 128, nq, 512).transpose(2, 1, 0, 3)
        ).reshape(nq, 128, kd * 512)

    def wblk(w):
        # [D, G] -> [128, (D//128)*G] with k-block at cols k*G:(k+1)*G
        return bf(np.ascontiguousarray(
            np.asarray(w).reshape(D // 128, 128, G).transpose(1, 0, 2)
        ).reshape(128, (D // 128) * G))

    scale = 1.0 / math.sqrt(DH)
    in_maps = []
    for c in range(N_CORES):
        b, g = c // 2, c % 2
        gs = slice(g * G, (g + 1) * G)
        in_maps.append({
            "xqT": bft(queries[b, :seq]),
            "xkT": bft(keys[b, :seq]),
            "xvT": bft(values[b, :seq]),
            "wqT": wblk(Wq[gs, :].T * scale),
            "wkT": wblk(Wk[gs, :].T),
            "wvT": wblk(Wv[gs, :].T),
            "wcT": bf(Wc[:, gs].T),
        })
    return in_maps


_NC_CACHE = {}


def run_cores(inputs, seq=L, trace=False, debug=False):
    key = (seq, debug)
    if key not in _NC_CACHE:
        _NC_CACHE[key] = build_nc(seq, debug=debug)
    nc = _NC_CACHE[key]
    in_maps = shard_inputs(
        inputs["keys"], inputs["queries"], inputs["values"],
        inputs["Wk"], inputs["Wq"], inputs["Wv"], inputs["Wc"], seq=seq)
    res = run_bass_kernel_spmd(nc, in_maps, core_ids=list(range(N_CORES)),
                               trace=trace)
    return res


def kernel(keys, queries, values, Wk, Wq, Wv, Wc, bc, attn_mask):
    res = run_cores(dict(keys=np.asarray(keys), queries=np.asarray(queries),
                         values=np.asarray(values), Wk=np.asarray(Wk),
                         Wq=np.asarray(Wq), Wv=np.asarray(Wv),
                         Wc=np.asarray(Wc)))
    bc = np.asarray(bc, np.float32)
    out = np.empty((B, L, D), np.float32)
    for b in range(B):
        out[b] = (res.results[2 * b]["outp"].astype(np.float32).sum(0)
                  + res.results[2 * b + 1]["outp"].astype(np.float32).sum(0)
                  ).T + bc
    return out

